# revision 2
# baseline (speedup 1.0000x reference)
"""Trainium2 Bass kernel v3 for nn_LoopVisibleLSTM (T=2048, B=32, D=256, H=256, L=2).

8-core batch-parallel (B=4 per core), transposed-gates design.
Per core, per step, per layer:
  - 16 recurrent matmuls [128h x 128gate] fp8-FWL stationary, bf16 h moving
    (N=4 batch cols) accumulating into a per-group PSUM bank.
  - one sigmoid [128, 32] over all 4 gate sections (stored order g i f o,
    g rows pre-doubled so tanh(z) = 2*sigmoid(2z)-1).
  - DVE cell update (fcb, igb, cst'), ACT tanh, and a single bf16 tap
    h = tanh(c')*sig_o written straight into the group h-buffer, which is
    simultaneously the next-step matmul rhs, the next-layer gin input, and
    (for layer 1) the output staging.
Input-side projections (gin) are bulk per 16-step group (one PSUM bank per
group per layer); biases ride in via K=8 one-hot matmuls.  Layer 1 lags
layer 0 by one group.  For_i hardware loop over 64-step bodies with peeled
first body and layer-1 epilogue.
"""

import sys
import os

for _p in ("/opt/pypackages", "/opt/trn_rl_repo"):
    if _p not in sys.path:
        sys.path.insert(0, _p)

import numpy as np

T_FULL, B_FULL, D, H = 2048, 32, 256, 256
NCORES = 8
B = B_FULL // NCORES      # batch per core = 4
G = 16                    # steps per group (one PSUM bank per group per layer)
PH = 16                   # groups per body
BODY = G * PH             # 256 steps per body
P_SC = 64.0               # PSUM gate pre-activation scale


def build(T):
    import concourse.bass as bass
    import concourse.mybir as mybir
    from concourse import bacc

    FP32 = mybir.dt.float32
    BF16 = mybir.dt.bfloat16
    F16 = mybir.dt.float16
    FP8 = mybir.dt.float8e3
    AF = mybir.ActivationFunctionType
    ALU = mybir.AluOpType

    assert T % BODY == 0
    n_body = T // BODY
    GB = G * B                     # 64 cols per group (q-block width)

    nc = bacc.Bacc("TRN2", target_bir_lowering=False, debug=False)

    # ---------------- DRAM ----------------
    inpT = nc.declare_dram_parameter("inputT", [256, (T + BODY) * B], BF16,
                                     isOutput=False)
    whh8_d = [nc.declare_dram_parameter(f"whh8_{l}", [128, 2048], FP8,
                                        isOutput=False) for l in range(2)]
    wih_d = [nc.declare_dram_parameter(f"wih_{l}", [128, 2048], BF16,
                                       isOutput=False) for l in range(2)]
    bias8_d = [nc.declare_dram_parameter(f"bias8_{l}", [8, 128], BF16,
                                         isOutput=False) for l in range(2)]
    onehot_d = nc.declare_dram_parameter("onehot8", [8, 512], BF16,
                                         isOutput=False)
    hinit_d = [nc.declare_dram_parameter(f"hinit_{l}", [128, 8], BF16,
                                         isOutput=False) for l in range(2)]
    cinit_d = [nc.declare_dram_parameter(f"cinit_{l}", [128, 8], FP32,
                                         isOutput=False) for l in range(2)]
    fwdT = nc.declare_dram_parameter("fwdT", [256, T * B], BF16, isOutput=True)

    ctxs = []

    def sb(shape, dtype=FP32):
        cm = nc.sbuf_tensor(shape, dtype)
        t = cm.__enter__()
        ctxs.append(cm)
        return t

    def ps(shape, dtype=FP32):
        cm = nc.psum_tensor(shape, dtype)
        t = cm.__enter__()
        ctxs.append(cm)
        return t

    # ---------------- SBUF ----------------
    whh8 = [sb([128, 2048], FP8) for _ in range(2)]
    wih = [sb([128, 2048], BF16) for _ in range(2)]
    scratch = sb([128, 8], FP32)
    bias8 = [sb([8, 128], BF16) for _ in range(2)]
    onehot = sb([8, 512], BF16)
    inT = [[sb([128, 128], BF16) for _ in range(2)] for _ in range(2)]  # [dk][slot]
    hgrp = [[sb([128, 128], BF16) for _ in range(2)] for _ in range(2)]  # [l][gpar]
    cst = [[sb([128, 8], FP32) for _ in range(2)] for _ in range(2)]     # [l][par]
    sbf = [[sb([128, 32], F16) for _ in range(2)] for _ in range(2)]
    igb = [[sb([128, 8], FP32) for _ in range(2)] for _ in range(2)]
    fcb = [[sb([128, 8], FP32) for _ in range(2)] for _ in range(2)]
    s2c = [[sb([128, 8], F16) for _ in range(2)] for _ in range(2)]

    # ---------------- PSUM: one bank per (layer, group parity) ----------------
    gp = [[ps([128, 512]) for _ in range(2)] for _ in range(2)]          # [l][gpar]

    import concourse.tile as tile_mod

    with tile_mod.TileContext(nc) as tc:
        dma = nc.sync

        for l in range(2):
            dma.dma_start(whh8[l][:, :], whh8_d[l][:, :])
            dma.dma_start(wih[l][:, :], wih_d[l][:, :])
            dma.dma_start(bias8[l][:, :], bias8_d[l][:, :])
            dma.dma_start(cst[l][0][:, :], cinit_d[l][:, :])
            # initial h goes where group "-1" step 15's tap would live
            dst = hgrp[l][1][:, :].rearrange("p (h s c) -> p h s c", h=2, s=G)
            dma.dma_start(dst[:, :, G - 1, :],
                          hinit_d[l][:, :].rearrange("p (h c) -> p h c", h=2))
        dma.dma_start(onehot[:, :], onehot_d[:, :])
        dma.dma_start(inT[0][0][:, :], inpT[0:128, 0:128])
        dma.dma_start(inT[1][0][:, :], inpT[128:256, 0:128])
        dma.dma_start(inT[0][1][:, :], inpT[0:128, 128:256])
        dma.dma_start(inT[1][1][:, :], inpT[128:256, 128:256])

        def emit_bias(l, gpar, half):
            # bias for cols [256*half, 256*half+256) of bank gp[l][gpar];
            # half 0 carries start=True (zeroes the whole bank)
            nc.tensor.matmul(
                gp[l][gpar][:, 256 * half:256 * (half + 1)],
                bias8[l][:, :],
                onehot[:, 256 * half:256 * (half + 1)],
                start=(half == 0), stop=False, skip_group_check=True,
            )

        def emit_gin0(gpar, dk, q, slot, col0):
            # layer-0 input projection, full q-block (N=64)
            nc.tensor.matmul(
                gp[0][gpar][:, GB * q:GB * (q + 1)],
                wih[0][:, 128 * (dk * 8 + q):128 * (dk * 8 + q + 1)],
                inT[dk][slot][:, col0:col0 + GB],
                start=False, stop=(dk == 1), skip_group_check=True,
            )

        def emit_gin1(gpar, dk, q, half):
            # layer-1 input projection from layer-0 taps, half-group (N=32)
            nc.tensor.matmul(
                gp[1][gpar][:, GB * q + 32 * half:GB * q + 32 * (half + 1)],
                wih[1][:, 128 * (dk * 8 + q):128 * (dk * 8 + q + 1)],
                hgrp[0][gpar][:, GB * dk + 32 * half:GB * dk + 32 * (half + 1)],
                start=False, stop=(dk == 1), skip_group_check=True,
            )

        def emit_rec(l, gpar, s):
            hbuf = hgrp[l][1 - gpar] if s == 0 else hgrp[l][gpar]
            hcol = 4 * (G - 1) if s == 0 else 4 * (s - 1)
            for q in range(8):
                for hc in range(2):
                    nc.tensor.matmul(
                        gp[l][gpar][:, GB * q + 4 * s:GB * q + 4 * s + 4],
                        whh8[l][:, 128 * (hc * 8 + q):128 * (hc * 8 + q + 1)],
                        hbuf[:, GB * hc + hcol:GB * hc + hcol + 4],
                        start=False, stop=(q == 7 and hc == 1),
                        skip_group_check=True,
                    )

        def emit_sig(l, gpar, s):
            p = s % 2
            gv = gp[l][gpar][:, :].rearrange("p (q c) -> p q c", q=8)
            sv = sbf[l][p][:, :].rearrange("p (q c) -> p q c", q=8)
            nc.scalar.activation(sv[:, :, :], gv[:, :, 4 * s:4 * s + 4],
                                 AF.Sigmoid, scale=1.0 / P_SC)

        def emit_cell(l, s):
            p = s % 2
            pn = (s + 1) % 2
            s_ = sbf[l][p]
            nc.vector.tensor_mul(fcb[l][p][:, :], s_[:, 16:24], cst[l][p][:, :])
            nc.vector.scalar_tensor_tensor(
                igb[l][p][:, :], s_[:, 0:8], 0.5, s_[:, 8:16],
                ALU.subtract, ALU.mult)
            nc.vector.scalar_tensor_tensor(
                cst[l][pn][:, :], igb[l][p][:, :], 2.0, fcb[l][p][:, :],
                ALU.mult, ALU.add)

        def emit_tanh(l, s):
            p = s % 2
            pn = (s + 1) % 2
            nc.scalar.activation(s2c[l][p][:, :], cst[l][pn][:, :], AF.Tanh)

        def emit_tap(l, gpar, s):
            p = s % 2
            dst = hgrp[l][gpar][:, :].rearrange("p (h c) -> p h c", h=2)[
                :, :, 4 * s:4 * s + 4]
            nc.vector.tensor_mul(
                dst,
                s2c[l][p][:, :].rearrange("p (h c) -> p h c", h=2),
                sbf[l][p][:, 24:32].rearrange("p (h c) -> p h c", h=2))

        def emit_out_dma(gpar, col):
            src = hgrp[1][gpar]
            dma.dma_start(fwdT[0:128, bass.ds(col, GB)], src[:, 0:GB])
            dma.dma_start(fwdT[128:256, bass.ds(col, GB)], src[:, GB:2 * GB])

        def emit_pin(idx, l, gpar, half, s):
            # tiny DVE read of the psum bank tied to this step's sigmoid
            # output: extends the bank's WAR chain so the scheduler cannot
            # hoist the next group's bias/gin lump to the phase boundary
            nc.vector.tensor_add(scratch[:, idx:idx + 1],
                                 gp[l][gpar][:, 256 * half:256 * half + 1],
                                 sbf[l][s % 2][:, 0:1])

        def emit_phase(i, ph, do_l0=True, do_l1=True, do_half1=True,
                       do_gin_next=True, do_gin1=True, do_out=True,
                       do_pref=True, do_pin0=True, do_pin1=True):
            """One 16-step phase: l0 steps group (4b+ph), l1 steps group
            (4b+ph-1).  i is the body's first DRAM column (For_i var or int).
            """
            g0par = ph % 2
            g1par = (ph + 1) % 2
            nslot = ((ph + 1) // 2) % 2
            ncol = GB * ((ph + 1) % 2)
            for s in range(G):
                # ---- PE extras first (fill tap-wait gaps) ----
                if do_half1 and 1 <= s <= 4:
                    for dk in range(2):
                        for q in (2 * (s - 1), 2 * (s - 1) + 1):
                            emit_gin1(g1par, dk, q, 1)
                if do_gin_next:
                    if s == 5:
                        emit_bias(0, (ph + 1) % 2, 0)
                    if s == 9:
                        emit_bias(0, (ph + 1) % 2, 1)
                    if 5 <= s <= 8:
                        for dk in range(2):
                            emit_gin0((ph + 1) % 2, dk, s - 5, nslot, ncol)
                    if 9 <= s <= 12:
                        for dk in range(2):
                            emit_gin0((ph + 1) % 2, dk, s - 5, nslot, ncol)
                if do_gin1:
                    if s == 3:
                        emit_bias(1, g0par, 0)
                    if s == 7:
                        emit_bias(1, g0par, 1)
                    if 10 <= s <= 13:
                        for dk in range(2):
                            for q in (2 * (s - 10), 2 * (s - 10) + 1):
                                emit_gin1(g0par, dk, q, 0)
                if do_out and s == 1:
                    emit_out_dma(ph % 2, i + GB * ph - 2 * GB)
                if do_pref and s == 6 and ph % 2 == 0:
                    # refill the slot two groups ahead (slots hold 2 groups)
                    slot = (ph // 2 + 1) % 2
                    dma.dma_start(inT[0][slot][:, :],
                                  inpT[0:128, bass.ds(i + GB * ph + 128, 128)])
                    dma.dma_start(inT[1][slot][:, :],
                                  inpT[128:256, bass.ds(i + GB * ph + 128, 128)])
                # ---- the two layer chains ----
                if do_l0:
                    emit_rec(0, g0par, s)
                if do_l1:
                    emit_rec(1, g1par, s)
                if do_l0:
                    emit_sig(0, g0par, s)
                if do_l1:
                    emit_sig(1, g1par, s)
                if do_l0:
                    emit_cell(0, s)
                if do_l1:
                    emit_cell(1, s)
                if do_l0:
                    emit_tanh(0, s)
                if do_l1:
                    emit_tanh(1, s)
                if do_l0:
                    emit_tap(0, g0par, s)
                if do_l1:
                    emit_tap(1, g1par, s)
                # pins last in the DVE queue: spread the next group's
                # bias/gin release across the phase without delaying the
                # chain trio
                if do_gin_next and do_pin0:
                    if s == 4:
                        emit_pin(0, 0, (ph + 1) % 2, 0, s)
                    if s == 8:
                        emit_pin(1, 0, (ph + 1) % 2, 1, s)
                if do_gin1 and do_pin1:
                    if s == 2:
                        emit_pin(2, 1, g0par, 0, s)
                    if s == 6:
                        emit_pin(3, 1, g0par, 1, s)

        # ---------------- prologue: body 0 ----------------
        emit_bias(0, 0, 0)
        emit_bias(0, 0, 1)
        for dk in range(2):
            for q in range(8):
                emit_gin0(0, dk, q, 0, 0)
        emit_phase(0, 0, do_l1=False, do_half1=False, do_out=False,
                   do_pref=False, do_pin0=False, do_pin1=False)
        emit_phase(0, 1, do_out=False, do_pin1=False)
        for ph in range(2, PH):
            emit_phase(0, ph, do_gin_next=(n_body > 1 or ph < PH - 1))

        # ---------------- steady-state bodies 1..n_body-1 ----------------
        # (the last body's final gin/prefetch read the zero-padded DRAM
        # region and write a never-read PSUM bank -- harmless)
        if n_body > 1:
            with tc.For_i(BODY * B, n_body * BODY * B, BODY * B) as i:
                for ph in range(PH):
                    emit_phase(i, ph)

        # ---------------- epilogue: layer-1 group (T/G - 1) ----------------
        emit_phase(T * B, 0, do_l0=False, do_gin_next=False, do_gin1=False,
                   do_pref=False)
        emit_out_dma(1, T * B - GB)

    for cm in reversed(ctxs):
        cm.__exit__(None, None, None)

    nc.compile()
    return nc


def prep_inputs(inputs, T, n_cores=NCORES):
    """Host-side reparameterization; returns one input map per core."""
    import ml_dtypes
    E3M4 = ml_dtypes.float8_e3m4
    BF16 = ml_dtypes.bfloat16

    Wih = inputs["Wih"].astype(np.float32)
    Whh = inputs["Whh"].astype(np.float32)
    bih = inputs["bih"].astype(np.float32)
    bhh = inputs["bhh"].astype(np.float32)
    W_init = inputs["W_init"].astype(np.float32)
    b_init = inputs["b_init"].astype(np.float32)
    h0 = inputs["h0"].astype(np.float32)
    c0 = inputs["c0"].astype(np.float32)
    inp = np.ascontiguousarray(inputs["input"], np.float32)
    b_full = inp.shape[1]

    # stored gate-section order [g i f o]; g rows doubled (tanh folding)
    perm = np.concatenate([np.arange(512, 768), np.arange(0, 256),
                           np.arange(256, 512), np.arange(768, 1024)])
    R = np.ones((1024, 1), np.float32)
    R[512:768] = 2.0

    shared = {}
    for l in range(2):
        if l == 0:
            Wihp = Wih[0] @ W_init
            biasp = bih[0] + bhh[0] + Wih[0] @ b_init
        else:
            Wihp = Wih[1]
            biasp = bih[1] + bhh[1]
        Wr = (P_SC * R * Wihp)[perm]               # [1024 stored, 256 d]
        Whr = (P_SC * R * Whh[l])[perm]            # [1024 stored, 256 h]

        wih_sb = np.empty((128, 2048), np.float32)
        whh_sb = np.empty((128, 2048), np.float32)
        for k in range(2):
            for q in range(8):
                wih_sb[:, 128 * (k * 8 + q):128 * (k * 8 + q + 1)] = \
                    Wr[128 * q:128 * (q + 1), 128 * k:128 * (k + 1)].T
                whh_sb[:, 128 * (k * 8 + q):128 * (k * 8 + q + 1)] = \
                    Whr[128 * q:128 * (q + 1), 128 * k:128 * (k + 1)].T
        shared[f"wih_{l}"] = wih_sb.astype(BF16)
        shared[f"whh8_{l}"] = whh_sb.astype(E3M4)

        bs = (P_SC * R[:, 0] * biasp)[perm]        # [1024]
        shared[f"bias8_{l}"] = bs.reshape(8, 128).astype(BF16)

    oh = np.zeros((8, 512), np.float32)
    for r in range(8):
        oh[r, G * B * r:G * B * (r + 1)] = 1.0
    shared["onehot8"] = oh.astype(BF16)

    maps = []
    for core in range(n_cores):
        bsl = slice(core * B, (core + 1) * B)
        im = dict(shared)
        # input.T [256, (T+BODY)*B] bf16, col = t*B + b
        ipT = np.zeros((256, (T + BODY) * B), dtype=BF16)
        ipT[:, :T * B] = inp[:, bsl, :].reshape(T * B, 256).T.astype(BF16)
        im["inputT"] = ipT
        for l in range(2):
            hi = np.empty((128, 8), np.float32)
            ci = np.empty((128, 8), np.float32)
            for hc in range(2):
                hi[:, 4 * hc:4 * hc + 4] = h0[l][bsl, 128 * hc:128 * (hc + 1)].T
                ci[:, 4 * hc:4 * hc + 4] = c0[l][bsl, 128 * hc:128 * (hc + 1)].T
            im[f"hinit_{l}"] = hi.astype(BF16)
            im[f"cinit_{l}"] = ci
        maps.append(im)
    return maps


def gather_output(results, T, n_cores=NCORES):
    """results: list of per-core {"fwdT": [256, T*B] bf16} -> [T, B_full, 2H]."""
    b_full = B * n_cores
    out = np.empty((T, b_full, 2 * H), dtype=np.float32)
    for core, r in enumerate(results):
        fwd = r["fwdT"].astype(np.float32).T.reshape(T, B, 256)
        out[:, core * B:(core + 1) * B, :H] = fwd
    out[:, :, H:] = out[-1:, :, :H]
    return out


def kernel(**inputs):
    from concourse import bass_utils
    T = inputs["input"].shape[0]
    nc = build(T)
    maps = prep_inputs(inputs, T)
    res = bass_utils.run_bass_kernel_spmd(nc, maps, list(range(NCORES)))
    return gather_output(res.results, T)


def np_ref(inputs, T):
    x_all = np.asarray(inputs["input"], np.float32)
    h = np.asarray(inputs["h0"], np.float32).copy()
    c = np.asarray(inputs["c0"], np.float32).copy()
    Wih = inputs["Wih"]; Whh = inputs["Whh"]
    bih = inputs["bih"]; bhh = inputs["bhh"]
    outs = []
    for t in range(T):
        x = x_all[t] @ inputs["W_init"].T + inputs["b_init"]
        for l in range(2):
            gates = x @ Wih[l].T + bih[l] + h[l] @ Whh[l].T + bhh[l]
            i_, f_, g_, o_ = np.split(gates, 4, axis=-1)
            i_ = 1 / (1 + np.exp(-i_)); f_ = 1 / (1 + np.exp(-f_))
            o_ = 1 / (1 + np.exp(-o_)); g_ = np.tanh(g_)
            c[l] = f_ * c[l] + i_ * g_
            h[l] = o_ * np.tanh(c[l])
            x = h[l]
        outs.append(h[1].copy())
    return np.stack(outs)


if __name__ == "__main__":
    from concourse.bass_interp import CoreSim

    T = int(os.environ.get("SIM_T", "64"))
    rng = np.random.default_rng(0)
    k = 1.0 / np.sqrt(H)
    BF = 32
    inputs = {
        "input": rng.standard_normal((T, BF, D), dtype=np.float32),
        "W_init": rng.uniform(-k, k, (H, D)).astype(np.float32),
        "b_init": rng.uniform(-k, k, (H,)).astype(np.float32),
        "Wih": rng.uniform(-k, k, (2, 4 * H, H)).astype(np.float32),
        "Whh": rng.uniform(-k, k, (2, 4 * H, H)).astype(np.float32),
        "bih": rng.uniform(-k, k, (2, 4 * H)).astype(np.float32),
        "bhh": rng.uniform(-k, k, (2, 4 * H)).astype(np.float32),
        "h0": rng.uniform(-k, k, (2, BF, H)).astype(np.float32),
        "c0": rng.uniform(-k, k, (2, BF, H)).astype(np.float32),
    }
    expected = np_ref(inputs, T)          # [T, BF, 256]

    nc = build(T)
    maps = prep_inputs(inputs, T)
    sim = CoreSim(nc, trace=os.environ.get("SIM_TRACE", "0") == "1")
    for name, arr in maps[0].items():
        sim.tensor(name)[:] = arr
    sim.simulate()
    fwd = np.asarray(sim.tensor("fwdT")).astype(np.float32).T.reshape(T, B, 256)
    exp0 = expected[:, 0:B, :]
    err = np.abs(fwd - exp0).max() / (np.abs(exp0).max() + 1e-9)
    print("SIM time ns:", sim.time, " ns/step:", sim.time / T)
    print("SIM max-rel err:", err)
    print("sample got", fwd[3, 0, :4], "exp", exp0[3, 0, :4])
